# revision 1
# baseline (speedup 1.0000x reference)
"""Causal attention block (QKV proj + RoPE + causal SDPA + out proj) on 8
Trainium2 NeuronCores.

Sharding: core c = 4*b + g handles batch b (of 2) and head group g (of 4,
4 heads each).  Each core computes q/k/v for its 4 heads from x[b] and the
matching Wqkv column slices, runs causal SDPA, and contracts its 512
output-feature rows of Wproj, producing a partial projT [2048, 2048].  The
host sums the 4 partials per batch (the "all-reduce") and transposes.

All matmuls run in float32r (fp32 data, 1 cycle/row on the PE when the
moving free dim >= 256; ~1.5e-4 relative error at K=256).

Device layouts (per core):
  xT    [C=2048, N=2048]  x[b] transposed (contraction dim C on partitions)
  wq/wk/wv [2048, 512]    Wqkv column slices for this head group
  wp    [512, 2048]       Wproj rows for this head group
  cosT  [128, 2048]       RoPE cos, head-dim on partitions
  sinT  [128, 2048]       RoPE sin, head-dim on partitions, first 64
                          partitions negated (folds rotate_half's sign)
  ones  [128, 128]        all-ones (rowsum via matmul)
  tri   [128, 128]        tri[j, i] = 1 if i >= j else 0 (causal diag mask)
  projT [2048, 2048] out  partial output, transposed

Inside: q^T/k^T computed per head as [hd=128, tok] (RoPE applied with
partition-half swap), v as [tok, hd].  Scores are computed transposed
(scT[j, i] = k_j . q_i) so softmax-exp tiles feed the attn@v matmul with no
transposes anywhere.  Softmax skips max-subtraction (|scores| <= ~8 here,
exp is safe in fp32); row sums come from an all-ones matmul and are divided
out after the attn@v accumulation.
"""

import sys

if "/opt/trn_rl_repo" not in sys.path:
    sys.path.insert(0, "/opt/trn_rl_repo")

from contextlib import ExitStack

import numpy as np

import concourse.bass as bass  # noqa: F401
import concourse.tile as tile
from concourse import bacc, bass_utils, mybir

F32 = mybir.dt.float32
F32R = mybir.dt.float32r
EXP = mybir.ActivationFunctionType.Exp

B, N, C = 2, 2048, 2048
H = 16  # total heads
HD = C // H  # 128
G = 4  # head groups (cores per batch)
HPG = H // G  # 4 heads per group
P = 128
PANEL = 512
NP = N // PANEL  # 4 token panels
KB = C // P  # 16 contraction blocks
SCALE = float(HD) ** -0.5
ROPE_BASE = 10000.0

_NC_CACHE = {}
DEBUG = False
REPS = 1
COMPUTE = True
PHASES = "ABC"
EXPBATCH = True


class _NoOpEngine:
    def __getattr__(self, name):
        return lambda *a, **k: None


def _emit(ctx, tc, t):
    nc = tc.nc
    const = ctx.enter_context(tc.tile_pool(name="const", bufs=1))
    xpool = ctx.enter_context(tc.tile_pool(name="x", bufs=2))
    qkv = ctx.enter_context(tc.tile_pool(name="qkv", bufs=1))
    epool = ctx.enter_context(tc.tile_pool(name="e", bufs=5))
    tmp = ctx.enter_context(tc.tile_pool(name="tmp", bufs=2))
    opool = ctx.enter_context(tc.tile_pool(name="o", bufs=1))
    pout = ctx.enter_context(tc.tile_pool(name="po", bufs=2))
    ps = ctx.enter_context(tc.tile_pool(name="ps", bufs=1, space="PSUM"))

    cosT = const.tile([P, N], F32)
    sinT = const.tile([P, N], F32)
    ones = const.tile([P, P], F32R)
    tri = const.tile([P, P], F32)
    nc.sync.dma_start(cosT, t["cosT"])
    nc.sync.dma_start(sinT, t["sinT"])
    nc.sync.dma_start(ones, t["ones"])
    nc.sync.dma_start(tri, t["tri"])

    outT = [
        opool.tile([P, N], F32R, tag=f"outT{h}", name=f"outT{h}") for h in range(HPG)
    ]

    xT3 = t["xT"].rearrange("(kb q) n -> q kb n", q=P)
    mm = nc.tensor.matmul

    if REPS == 1:
        _emit_once(tc, t, const, xpool, qkv, epool, tmp, opool, pout, ps,
                   cosT, sinT, ones, tri, outT, xT3, mm)
    else:
        with tc.For_i(0, REPS, 1):
            _emit_once(tc, t, const, xpool, qkv, epool, tmp, opool, pout, ps,
                       cosT, sinT, ones, tri, outT, xT3, mm)


def _emit_once(tc, t, const, xpool, qkv, epool, tmp, opool, pout, ps,
               cosT, sinT, ones, tri, outT, xT3, mm):
    nc = tc.nc
    vec = nc.vector if COMPUTE else _NoOpEngine()
    sca = nc.scalar if COMPUTE else _NoOpEngine()
    if not COMPUTE:
        mm = lambda *a, **k: None  # noqa: E731

    # wp (proj weights) shares the x pool slots (16KB each), loaded as halves
    wp_half = [None, None]

    def load_wp():
        wp3 = t["wp"].rearrange("(h p) o -> p h o", p=P)
        for i in range(2):
            wp_half[i] = xpool.tile([P, 2, N], F32R, tag="x", name=f"wp{i}")
            nc.sync.dma_start(wp_half[i], wp3[:, 2 * i : 2 * i + 2, :])

    def wp_block(h, obs):
        # lhsT tile [128, 128] for local head h, output block ob
        return wp_half[h // 2][:, h % 2, 128 * obs : 128 * (obs + 1)]

    def emit_proj_panel(p):
        sl = slice(PANEL * p, PANEL * (p + 1))
        for ob in range(KB):
            pj = ps.tile(
                [P, PANEL], F32, tag=f"V{2 + (ob % 2)}", name="pj"
            )
            for h in range(HPG):
                mm(
                    pj,
                    wp_block(h, ob),
                    outT[h][:, sl],
                    start=(h == 0),
                    stop=(h == HPG - 1),
                )
            if COMPUTE:
                o_t = pout.tile([P, PANEL], F32, tag="pout")
                if ob % 2 == 0:
                    sca.copy(o_t, pj)
                else:
                    vec.tensor_copy(o_t, pj)
                nc.sync.dma_start(t["projT"][128 * ob : 128 * (ob + 1), sl], o_t)
            else:
                nc.sync.dma_start(
                    t["projT"][128 * ob : 128 * (ob + 1), sl], cosT[:, 0:PANEL]
                )

    with tc.tile_pool(name="w", bufs=1) as wpool, tc.tile_pool(
        name="qkraw", bufs=2
    ) as rawpool:
        for sweep in range(2):
            # ---- phase A: QKV + RoPE for heads (2*sweep, 2*sweep+1) ----
            w_sb = {}
            for wname in ("wq", "wk", "wv"):
                w_sb[wname] = wpool.tile([P, KB, 256], F32R, tag=wname, name=wname)
                nc.sync.dma_start(
                    w_sb[wname],
                    t[wname].rearrange("(kb p) f -> p kb f", p=P)[
                        :, :, 256 * sweep : 256 * sweep + 256
                    ],
                )
            v_sb = qkv.tile([P, KB, 256], F32R, tag="v")
            qk = {}
            for hh in range(2):
                qk["q", hh] = qkv.tile([P, N], F32R, tag=f"q{hh}", name=f"q{hh}")
                qk["k", hh] = qkv.tile([P, N], F32R, tag=f"k{hh}", name=f"k{hh}")

            for p in range(NP):
                sl = slice(PANEL * p, PANEL * (p + 1))
                pq = [
                    ps.tile([P, PANEL], F32, tag=f"A{i}", name=f"pq{i}")
                    for i in range(2)
                ]
                pk = [
                    ps.tile([P, PANEL], F32, tag=f"A{i + 2}", name=f"pk{i}")
                    for i in range(2)
                ]
                pv = [
                    ps.tile([P, 256], F32, tag=f"V{tb}", name=f"pv{tb}")
                    for tb in range(4)
                ]
                for hb in range(2):
                    xt = xpool.tile([P, KB // 2, PANEL], F32R, tag="x")
                    nc.sync.dma_start(xt, xT3[:, 8 * hb : 8 * hb + 8, sl])
                    for kbl in range(KB // 2):
                        kb = 8 * hb + kbl
                        st, sp = kb == 0, kb == KB - 1
                        x_k = xt[:, kbl]
                        mm(pq[0], w_sb["wq"][:, kb, 0:128], x_k, start=st, stop=sp)
                        mm(pq[1], w_sb["wq"][:, kb, 128:256], x_k, start=st, stop=sp)
                        mm(pk[0], w_sb["wk"][:, kb, 0:128], x_k, start=st, stop=sp)
                        mm(pk[1], w_sb["wk"][:, kb, 128:256], x_k, start=st, stop=sp)
                        for tb in range(4):
                            mm(
                                pv[tb],
                                x_k[:, 128 * tb : 128 * (tb + 1)],
                                w_sb["wv"][:, kb],
                                start=st,
                                stop=sp,
                            )
                # Fast ACT copies free the q/k psum banks; RoPE runs on DVE
                # from SBUF off the critical path.
                # rope(q) = q*cos + swap64(q)*sin' (sin' pre-signed)
                for psrc, dst in (
                    (pq[0], qk["q", 0]),
                    (pq[1], qk["q", 1]),
                    (pk[0], qk["k", 0]),
                    (pk[1], qk["k", 1]),
                ):
                    raws = rawpool.tile([P, PANEL], F32, tag="raws")
                    rawsw = rawpool.tile([P, PANEL], F32, tag="rawsw")
                    sca.copy(raws, psrc)
                    sca.copy(rawsw[0:64], psrc[64:128])
                    sca.copy(rawsw[64:128], psrc[0:64])
                    t1 = tmp.tile([P, PANEL], F32, tag="rope1")
                    t2 = tmp.tile([P, PANEL], F32, tag="rope2")
                    vec.tensor_mul(t1, rawsw, sinT[:, sl])
                    vec.tensor_mul(t2, raws, cosT[:, sl])
                    vec.tensor_add(dst[:, sl], t2, t1)
                for tb in range(4):
                    sca.copy(v_sb[:, 4 * p + tb, :], pv[tb])

            # ---- phase B: causal SDPA, both heads; proj inlined on sweep 1
            if sweep == 1 and "C" in PHASES:
                load_wp()
            for p in range(NP if "B" in PHASES else 0):
                sl = slice(PANEL * p, PANEL * (p + 1))
                po = {}
                prs = {}
                e_tiles = {0: [], 1: []}
                for hh in range(2):
                    po[hh] = ps.tile(
                        [P, PANEL], F32, tag=f"V{hh}", name=f"po{hh}"
                    )
                    prs[hh] = ps.tile(
                        [P, PANEL], F32, tag=f"V{2 + hh}", name=f"prs{hh}"
                    )
                njb = 4 * p + 4

                def emit_av(hh, jj):
                    e_t, n0 = e_tiles[hh][jj]
                    st, sp = jj == 0, jj == njb - 1
                    mm(
                        po[hh][:, n0:],
                        v_sb[:, jj, 128 * hh : 128 * hh + 128],
                        e_t[:, n0:],
                        start=st,
                        stop=sp,
                    )
                    mm(prs[hh][:, n0:], ones, e_t[:, n0:], start=st, stop=sp)

                for jb in range(njb):
                    td = jb - 4 * p  # diagonal sub-block index if >= 0
                    n0 = 128 * td if td > 0 else 0
                    for hh in range(2):
                        if jb >= 3:
                            emit_av(hh, jb - 3)
                        sc1 = ps.tile(
                            [P, PANEL],
                            F32,
                            tag=f"A{(2 * jb + hh) % 4}",
                            name="sc1",
                        )
                        mm(
                            sc1[:, n0:],
                            qk["k", hh][:, 128 * jb : 128 * (jb + 1)],
                            qk["q", hh][:, PANEL * p + n0 : PANEL * (p + 1)],
                        )
                        e1 = epool.tile([P, PANEL], F32R, tag="e1")
                        sca.activation(e1[:, n0:], sc1[:, n0:], EXP, scale=SCALE)
                        if td >= 0:
                            dsl = slice(128 * td, 128 * (td + 1))
                            vec.tensor_mul(
                                e1[:, dsl], e1[:, dsl].bitcast(F32), tri
                            )
                        e_tiles[hh].append((e1, n0))
                for hh in range(2):
                    for jj in range(max(0, njb - 3), njb):
                        emit_av(hh, jj)
                    recip = tmp.tile([P, PANEL], F32, tag="rope1")
                    vec.reciprocal(recip, prs[hh])
                    vec.tensor_mul(
                        outT[2 * sweep + hh][:, sl], po[hh], recip
                    )
                if sweep == 1 and "C" in PHASES:
                    # proj for this panel: outT[0..3][:, sl] are all final now
                    emit_proj_panel(p)

    if DEBUG:
        for h in range(HPG):
            nc.sync.dma_start(t[f"dbg_o{h}"], outT[h].bitcast(F32))



def build_nc():
    key = (REPS, DEBUG, COMPUTE, PHASES, EXPBATCH)
    if key in _NC_CACHE:
        return _NC_CACHE[key]
    nc = bacc.Bacc("TRN2", target_bir_lowering=False, debug=False)
    t = {}
    t["xT"] = nc.dram_tensor("xT", [C, N], F32R, kind="ExternalInput").ap()
    t["wq"] = nc.dram_tensor("wq", [C, 512], F32R, kind="ExternalInput").ap()
    t["wk"] = nc.dram_tensor("wk", [C, 512], F32R, kind="ExternalInput").ap()
    t["wv"] = nc.dram_tensor("wv", [C, 512], F32R, kind="ExternalInput").ap()
    t["wp"] = nc.dram_tensor("wp", [512, N], F32R, kind="ExternalInput").ap()
    t["cosT"] = nc.dram_tensor("cosT", [P, N], F32, kind="ExternalInput").ap()
    t["sinT"] = nc.dram_tensor("sinT", [P, N], F32, kind="ExternalInput").ap()
    t["ones"] = nc.dram_tensor("ones", [P, P], F32R, kind="ExternalInput").ap()
    t["tri"] = nc.dram_tensor("tri", [P, P], F32, kind="ExternalInput").ap()
    t["projT"] = nc.dram_tensor("projT", [N, N], F32, kind="ExternalOutput").ap()
    if DEBUG:
        for h in range(HPG):
            t[f"dbg_q{h}"] = nc.dram_tensor(
                f"dbg_q{h}", [P, N], F32, kind="ExternalOutput"
            ).ap()
            t[f"dbg_k{h}"] = nc.dram_tensor(
                f"dbg_k{h}", [P, N], F32, kind="ExternalOutput"
            ).ap()
            t[f"dbg_o{h}"] = nc.dram_tensor(
                f"dbg_o{h}", [P, N], F32, kind="ExternalOutput"
            ).ap()
        for s in range(2):
            t[f"dbg_v{s}"] = nc.dram_tensor(
                f"dbg_v{s}", [N, 256], F32, kind="ExternalOutput"
            ).ap()
    with tile.TileContext(nc) as tc, ExitStack() as ctx:
        _emit(ctx, tc, t)
    nc.compile()
    _NC_CACHE[key] = nc
    return nc


def make_in_maps(x, position_ids, Wqkv, Wproj):
    x = np.asarray(x, dtype=np.float32)
    pos = np.asarray(position_ids, dtype=np.float64)
    Wqkv = np.asarray(Wqkv, dtype=np.float32)
    Wproj = np.asarray(Wproj, dtype=np.float32)

    inv_freq = 1.0 / (
        ROPE_BASE ** (np.arange(0, HD, 2, dtype=np.float32) / HD)
    )  # [64]
    ones = np.ones((P, P), dtype=np.float32)
    tri = (np.arange(P)[None, :] >= np.arange(P)[:, None]).astype(np.float32)

    in_maps = []
    for c in range(8):
        b, g = divmod(c, G)
        freqs = pos[b].astype(np.float32)[:, None] * inv_freq[None, :]  # [N, 64]
        emb = np.concatenate([freqs, freqs], axis=-1)  # [N, 128]
        cosT = np.ascontiguousarray(np.cos(emb).T)  # [128, N]
        sinT = np.sin(emb)
        sinT = np.ascontiguousarray(sinT.T)
        sinT[:64] = -sinT[:64]
        in_maps.append(
            {
                "xT": np.ascontiguousarray(x[b].T),
                "wq": np.ascontiguousarray(Wqkv[:, 512 * g : 512 * (g + 1)]),
                "wk": np.ascontiguousarray(
                    Wqkv[:, 2048 + 512 * g : 2048 + 512 * (g + 1)]
                ),
                "wv": np.ascontiguousarray(
                    Wqkv[:, 4096 + 512 * g : 4096 + 512 * (g + 1)]
                ),
                "wp": np.ascontiguousarray(Wproj[512 * g : 512 * (g + 1), :]),
                "cosT": cosT,
                "sinT": sinT,
                "ones": ones,
                "tri": tri,
            }
        )
    return in_maps


def kernel(x, position_ids, Wqkv, Wproj, _trace=False, _tmpdir=None):
    nc = build_nc()
    in_maps = make_in_maps(x, position_ids, Wqkv, Wproj)
    res = bass_utils.run_bass_kernel_spmd(
        nc, in_maps, core_ids=list(range(8)), trace=_trace, tmpdir=_tmpdir
    )
    out = np.empty((B, N, C), dtype=np.float32)
    for b in range(B):
        acc = res.results[4 * b]["projT"].copy()
        for g in range(1, G):
            acc += res.results[4 * b + g]["projT"]
        out[b] = acc.T
    kernel.last_exec_time_ns = res.exec_time_ns
    kernel.last_results = res
    return out



# revision 6
# speedup vs baseline: 1.2641x; 1.2641x over previous
"""Causal attention block (QKV proj + RoPE + causal SDPA + out proj) on 8
Trainium2 NeuronCores.

Sharding: core c = 4*b + g handles batch b (of 2) and head group g (of 4,
4 heads each).  Each core computes q/k/v for its 4 heads from x[b] and the
matching Wqkv column slices, runs causal SDPA, and contracts its 512
output-feature rows of Wproj, producing a partial projB [2048(tok),
2048(oc)].  The host sums the 4 partials per batch.

v2 design notes (vs the fp32r baseline):
  * All matmul operands are bf16 (PSUM accumulation stays fp32).  bf16
    stationaries enable Fast Weight Load (2 cols/cycle) -- fp32r LDWEIGHTS
    measured ~190ns/tile and made QKV LDW-port-bound (~224ns/MM observed vs
    160ns stream-ideal).  End-to-end bf16 error measured 3.9e-3 <= 2e-2.
  * Softmax row sums are fused into the attn@v matmuls: e-tiles are the
    STATIONARY operand ([128 keys, 128 queries] chunks) and the moving
    operand is v with an appended all-ones column [128 keys, 129].  The
    PSUM result is [queries, hd | rowsum], so the denominator lands as a
    per-partition scalar: reciprocal on [128,1] + tensor_scalar broadcast.
    This removes the separate all-ones rowsum matmul (1/3 of attention
    matmul rows in the baseline).
  * The [q, hd] attention output is transposed back to [hd, q] for the
    projection with SBUF->SBUF dma_start_transpose (xbar), costing no
    engine time.
  * Projection emits projB[tok, oc] (stationary = outT chunk, moving = Wproj
    rows), so neither device nor host transposes the output; output is bf16
    (halves the output DMA).
  * exp runs once per (panel, jb) over both heads' scores ([128, 2, 512-n0]
    strided PSUM read) halving ACT instruction overheads.
  * RoPE reads q/k PSUM directly (swap-halves via partition-offset ACT
    copies + in-place DVE muls); q/k/v PSUM banks free early so the next
    panel's matmuls are never blocked on the RoPE chain.
"""

import sys

if "/opt/trn_rl_repo" not in sys.path:
    sys.path.insert(0, "/opt/trn_rl_repo")

from contextlib import ExitStack

import ml_dtypes
import numpy as np

import concourse.bass as bass  # noqa: F401
import concourse.tile as tile
from concourse import bacc, bass_utils, mybir

F32 = mybir.dt.float32
BF16 = mybir.dt.bfloat16
EXP = mybir.ActivationFunctionType.Exp

B, N, C = 2, 2048, 2048
H = 16  # total heads
HD = C // H  # 128
G = 4  # head groups (cores per batch)
HPG = H // G  # 4 heads per group
P = 128
PANEL = 512
NP = N // PANEL  # 4 token panels
KB = C // P  # 16 contraction blocks
NJB = N // P  # 16 key blocks
SCALE = float(HD) ** -0.5
ROPE_BASE = 10000.0

_NC_CACHE = {}
DEBUG = False


def _bc2(ap, n=2):
    """Broadcast a [128, F] AP across an inserted middle dim -> [128, n, F]."""
    p, f = ap.shape
    return ap.rearrange("p (o n) -> p o n", o=1).broadcast_to([p, n, f])


def _emit(ctx, tc, t):
    nc = tc.nc
    vec = nc.vector
    sca = nc.scalar
    mm = nc.tensor.matmul

    const = ctx.enter_context(tc.tile_pool(name="const", bufs=1))
    wpool = ctx.enter_context(tc.tile_pool(name="w", bufs=2))
    xpool = ctx.enter_context(tc.tile_pool(name="x", bufs=2))
    qkpool = ctx.enter_context(tc.tile_pool(name="qk", bufs=2))
    vpool = ctx.enter_context(tc.tile_pool(name="v", bufs=2))
    rpool = ctx.enter_context(tc.tile_pool(name="rope", bufs=2))
    epool = ctx.enter_context(tc.tile_pool(name="e", bufs=4))
    opool = ctx.enter_context(tc.tile_pool(name="o", bufs=1))
    pnpool = ctx.enter_context(tc.tile_pool(name="pn", bufs=2))
    rspool = ctx.enter_context(tc.tile_pool(name="rs", bufs=4))
    poutp = ctx.enter_context(tc.tile_pool(name="pout", bufs=4))
    ps = ctx.enter_context(tc.tile_pool(name="ps", bufs=1, space="PSUM"))

    cosT = const.tile([P, N], F32)
    sinT = const.tile([P, N], F32)
    tri = const.tile([P, P], BF16)
    zeros = const.tile([P, 264], BF16)
    nc.sync.dma_start(cosT, t["cosT"])
    nc.sync.dma_start(sinT, t["sinT"])
    nc.sync.dma_start(tri, t["tri"])
    vec.memset(zeros, 0.0)

    # wp4 loaded later (during sweep 0) to keep the startup DMA short
    wp4 = const.tile([P, HPG, N], BF16, name="wp4")

    outT = [
        opool.tile([P, N], BF16, tag=f"outT{h}", name=f"outT{h}")
        for h in range(HPG)
    ]

    xT3 = t["xT"].rearrange("(kb q) n -> q kb n", q=P)

    def load_w(sweep):
        """Per-sweep 256-col slices of wq/wk/wv -> [128, KB, 256] bf16."""
        w_sb = {}
        for wname in ("wq", "wk", "wv"):
            w3 = t[wname].rearrange("(kb p) f -> p kb f", p=P)
            # split into halves so the first matmuls can start early
            w_t = wpool.tile([P, KB, 256], BF16, tag=wname, name=wname)
            for hf in range(2):
                nc.sync.dma_start(
                    w_t[:, 8 * hf : 8 * hf + 8, :],
                    w3[:, 8 * hf : 8 * hf + 8, 256 * sweep : 256 * sweep + 256],
                )
            w_sb[wname] = w_t
        return w_sb

    for sweep in range(2):
        w_sb = load_w(sweep) if sweep == 0 else w_next  # noqa: F821
        # per-sweep state
        qq = qkpool.tile([P, 2, N], BF16, tag="qq", name="qq")
        kk = qkpool.tile([P, 2, N], BF16, tag="kk", name="kk")
        v_sb = vpool.tile([P, NJB, 2, 132], BF16, tag="v", name="v_sb")
        vec.memset(v_sb[:, :, :, 128:129], 1.0)

        # ---- phase A: QKV + RoPE for this sweep's 2 heads ----
        for p in range(NP):
            sl = slice(PANEL * p, PANEL * (p + 1))
            pq01 = ps.tile([P, 2, PANEL], F32, tag="SC0", name="pq01")
            pk01 = ps.tile([P, 2, PANEL], F32, tag="SC1", name="pk01")
            pv = [
                ps.tile([P, 256], F32, tag=f"PO{tb}", name=f"pv{tb}")
                for tb in range(4)
            ]
            for hb in range(2):
                xt = xpool.tile([P, KB // 2, PANEL], BF16, tag="x")
                nc.sync.dma_start(xt, xT3[:, 8 * hb : 8 * hb + 8, sl])
                for kbl in range(KB // 2):
                    kb = 8 * hb + kbl
                    st, sp = kb == 0, kb == KB - 1
                    x_k = xt[:, kbl]
                    mm(pq01[:, 0], w_sb["wq"][:, kb, 0:128], x_k, start=st, stop=sp)
                    mm(pq01[:, 1], w_sb["wq"][:, kb, 128:256], x_k, start=st, stop=sp)
                    mm(pk01[:, 0], w_sb["wk"][:, kb, 0:128], x_k, start=st, stop=sp)
                    mm(pk01[:, 1], w_sb["wk"][:, kb, 128:256], x_k, start=st, stop=sp)
                    for tb in range(4):
                        mm(
                            pv[tb],
                            x_k[:, 128 * tb : 128 * (tb + 1)],
                            w_sb["wv"][:, kb],
                            start=st,
                            stop=sp,
                        )
            # RoPE: dst = psum*cos + swap64(psum)*sin'  (sin' pre-negated on
            # partitions 0-63).  cos/sin broadcast across the 2-head dim.
            for psrc, dst in ((pq01, qq), (pk01, kk)):
                rw = rpool.tile([P, 2, PANEL], F32, tag="rw")
                sca.copy(rw[0:64], psrc[64:128])
                sca.copy(rw[64:128], psrc[0:64])
                vec.tensor_mul(rw, rw, _bc2(sinT[:, sl]))
                t2 = rpool.tile([P, 2, PANEL], F32, tag="t2")
                vec.tensor_mul(t2, psrc, _bc2(cosT[:, sl]))
                vec.tensor_add(dst[:, :, sl], t2, rw)
            for tb in range(4):
                sca.copy(
                    v_sb[:, 4 * p + tb, :, 0:128],
                    pv[tb].rearrange("p (h f) -> p h f", h=2),
                )

        # prefetch next sweep's weights / wp4 during attention
        if sweep == 0:
            w_next = load_w(1)
            nc.sync.dma_start(wp4, t["wp4"])

        # ---- phase B: causal SDPA (+ proj on sweep 1) ----
        for p in range(NP):
            njb = 4 * p + 4
            po = {
                (hh, pair): ps.tile(
                    [P, 2, 132], F32, tag=f"PO{2 * hh + pair}", name="po"
                )
                for hh in range(2)
                for pair in range(2)
            }
            # Two accumulation groups share each po bank, but a start=True
            # matmul clears has_written for the WHOLE bank.  So pre-zero the
            # bank with one dummy matmul (sets has_written everywhere) and
            # accumulate with start=False.
            for key in po:
                mm(
                    po[key].rearrange("p a b -> p (a b)"),
                    tri,
                    zeros,
                    start=True,
                    stop=False,
                    skip_group_check=True,
                )
            e_tiles = []

            def emit_av(jj):
                e1, td = e_tiles[jj]
                for hh in range(2):
                    for qc in range(max(0, td), 4):
                        mm(
                            po[hh, qc // 2][:, qc % 2, 0:129],
                            e1[:, hh, 128 * qc : 128 * (qc + 1)],
                            v_sb[:, jj, hh, 0:129],
                            start=False,
                            stop=(jj == 4 * p + qc),
                            skip_group_check=True,
                        )

            for jj in range(njb):
                td = jj - 4 * p  # diagonal sub-block index if >= 0
                n0 = 128 * td if td > 0 else 0
                sc = ps.tile([P, 2, PANEL], F32, tag=f"SC{jj % 2}", name="sc")
                for hh in range(2):
                    mm(
                        sc[:, hh, n0:],
                        kk[:, hh, 128 * jj : 128 * (jj + 1)],
                        qq[:, hh, PANEL * p + n0 : PANEL * (p + 1)],
                    )
                e1 = epool.tile([P, 2, PANEL], BF16, tag="e1")
                sca.activation(e1[:, :, n0:], sc[:, :, n0:], EXP, scale=SCALE)
                if td >= 0:
                    dsl = slice(128 * td, 128 * (td + 1))
                    vec.tensor_mul(e1[:, :, dsl], e1[:, :, dsl], _bc2(tri))
                e_tiles.append((e1, td))
                if jj >= 2:
                    emit_av(jj - 2)
            emit_av(njb - 2)
            emit_av(njb - 1)

            # normalize by the fused rowsum (col 128) and transpose back
            for hh in range(2):
                po_n = pnpool.tile([P, PANEL], BF16, tag=f"pn{hh}", name="po_n")
                for qc in range(4):
                    src = po[hh, qc // 2][:, qc % 2]
                    rs_rec = rspool.tile([P, 1], F32, tag="rs")
                    vec.reciprocal(rs_rec, src[:, 128:129])
                    vec.tensor_scalar_mul(
                        po_n[:, 128 * qc : 128 * (qc + 1)], src[:, 0:128], rs_rec
                    )
                    nc.sync.dma_start_transpose(
                        outT[2 * sweep + hh][
                            :, PANEL * p + 128 * qc : PANEL * p + 128 * (qc + 1)
                        ],
                        po_n[:, 128 * qc : 128 * (qc + 1)],
                    )

            if sweep == 1:
                # proj for this panel: outT[0..3][:, psl] are final now
                for tc in range(4):
                    tsl = slice(PANEL * p + 128 * tc, PANEL * p + 128 * (tc + 1))
                    for occ in range(4):
                        pj = ps.tile(
                            [P, PANEL], F32, tag=f"SC{occ % 2}", name="pj"
                        )
                        osl = slice(PANEL * occ, PANEL * (occ + 1))
                        for h in range(HPG):
                            mm(
                                pj,
                                outT[h][:, tsl],
                                wp4[:, h, osl],
                                start=(h == 0),
                                stop=(h == HPG - 1),
                            )
                        o_t = poutp.tile([P, PANEL], BF16, tag="pout")
                        if occ % 2 == 0:
                            sca.copy(o_t, pj)
                        else:
                            vec.tensor_copy(o_t, pj)
                        nc.sync.dma_start(t["projB"][tsl, osl], o_t)

    if DEBUG:
        for h in range(HPG):
            nc.sync.dma_start(t[f"dbg_o{h}"], outT[h])


def build_nc():
    key = (DEBUG,)
    if key in _NC_CACHE:
        return _NC_CACHE[key]
    nc = bacc.Bacc("TRN2", target_bir_lowering=False, debug=False)
    t = {}
    t["xT"] = nc.dram_tensor("xT", [C, N], BF16, kind="ExternalInput").ap()
    t["wq"] = nc.dram_tensor("wq", [C, 512], BF16, kind="ExternalInput").ap()
    t["wk"] = nc.dram_tensor("wk", [C, 512], BF16, kind="ExternalInput").ap()
    t["wv"] = nc.dram_tensor("wv", [C, 512], BF16, kind="ExternalInput").ap()
    t["wp4"] = nc.dram_tensor("wp4", [P, HPG, N], BF16, kind="ExternalInput").ap()
    t["cosT"] = nc.dram_tensor("cosT", [P, N], F32, kind="ExternalInput").ap()
    t["sinT"] = nc.dram_tensor("sinT", [P, N], F32, kind="ExternalInput").ap()
    t["tri"] = nc.dram_tensor("tri", [P, P], BF16, kind="ExternalInput").ap()
    t["projB"] = nc.dram_tensor("projB", [N, N], BF16, kind="ExternalOutput").ap()
    if DEBUG:
        for h in range(HPG):
            t[f"dbg_o{h}"] = nc.dram_tensor(
                f"dbg_o{h}", [P, N], BF16, kind="ExternalOutput"
            ).ap()
    with tile.TileContext(nc) as tc, ExitStack() as ctx:
        _emit(ctx, tc, t)
    nc.compile()
    _NC_CACHE[key] = nc
    return nc


def make_in_maps(x, position_ids, Wqkv, Wproj):
    x = np.asarray(x, dtype=np.float32)
    pos = np.asarray(position_ids, dtype=np.float64)
    Wqkv = np.asarray(Wqkv, dtype=np.float32)
    Wproj = np.asarray(Wproj, dtype=np.float32)
    bf = ml_dtypes.bfloat16

    inv_freq = 1.0 / (
        ROPE_BASE ** (np.arange(0, HD, 2, dtype=np.float32) / HD)
    )  # [64]
    tri = (np.arange(P)[None, :] >= np.arange(P)[:, None]).astype(bf)

    in_maps = []
    for c in range(8):
        b, g = divmod(c, G)
        freqs = pos[b].astype(np.float32)[:, None] * inv_freq[None, :]  # [N, 64]
        emb = np.concatenate([freqs, freqs], axis=-1)  # [N, 128]
        cosT = np.ascontiguousarray(np.cos(emb).T)  # [128, N]
        sinT = np.ascontiguousarray(np.sin(emb).T)
        sinT[:64] = -sinT[:64]
        wp4 = np.ascontiguousarray(
            Wproj[512 * g : 512 * (g + 1), :]
            .reshape(HPG, P, N)
            .transpose(1, 0, 2)
        ).astype(bf)
        in_maps.append(
            {
                "xT": np.ascontiguousarray(x[b].T).astype(bf),
                "wq": np.ascontiguousarray(
                    Wqkv[:, 512 * g : 512 * (g + 1)]
                ).astype(bf),
                "wk": np.ascontiguousarray(
                    Wqkv[:, 2048 + 512 * g : 2048 + 512 * (g + 1)]
                ).astype(bf),
                "wv": np.ascontiguousarray(
                    Wqkv[:, 4096 + 512 * g : 4096 + 512 * (g + 1)]
                ).astype(bf),
                "wp4": wp4,
                "cosT": cosT,
                "sinT": sinT,
                "tri": tri,
            }
        )
    return in_maps


def kernel(x, position_ids, Wqkv, Wproj, _trace=False, _tmpdir=None):
    nc = build_nc()
    in_maps = make_in_maps(x, position_ids, Wqkv, Wproj)
    res = bass_utils.run_bass_kernel_spmd(
        nc, in_maps, core_ids=list(range(8)), trace=_trace, tmpdir=_tmpdir
    )
    out = np.empty((B, N, C), dtype=np.float32)
    for b in range(B):
        acc = res.results[4 * b]["projB"].astype(np.float32)
        for g in range(1, G):
            acc += res.results[4 * b + g]["projB"].astype(np.float32)
        out[b] = acc
    kernel.last_exec_time_ns = res.exec_time_ns
    kernel.last_results = res
    return out
